# revision 4
# baseline (speedup 1.0000x reference)
"""TRN2 Bass kernel for nn_DivTree (moe_routing): per-agent 2-layer MLP.

Math (per batch row b, agent a, with r = routing[a]):
    x0   = concat(x_in[b, a], onehot(a))                  # [H + A]
    h    = relu(x0 @ W1[r] + b1[r])                       # [H]
    out  = h @ W2[r] + b2[r]                              # [NACT]

Host-side simplifications baked in before the device kernel runs:
  - The onehot half of x0 @ W1[r] just selects row H+a of W1[r], so it is
    folded into an effective bias:  bias1e[a] = b1[r] + W1[r, H+a, :].
  - Expert weights are gathered by routing on the host (pure indexing).

Sharding: expert-parallel over agents. 48 agents are assigned whole to
cores (6 each); the remaining 2 agents are split into 4 batch-quarters
each (cores 0-3 take agent 48's quarters, cores 4-7 agent 49's), so all
8 cores run an identical program over 25 (agent, batch-512) work units.

Device kernel per unit (all matmuls in float32r — 1 cycle/row on PE):
    hT[m]  = relu(sum_k W1e[k, m-chunk].T @ xT[k-chunk] + bias1e)  # [128, 512] x4
    outT   = sum_k W2e[k-chunk].T @ hT[k] + b2e                    # [64, 512]
xT tiles are produced on the host (layout change during sharding), so the
device does pure matmul + activation work at the memory/compute ridge.
"""

import os
import sys

import numpy as np

sys.path.insert(0, "/opt/trn_rl_repo")

B, A, H, NACT = 2048, 50, 512, 64
N_CORES = 8
BT = 512  # batch tile (rows per work unit)
FULL_PER_CORE = 6  # whole agents per core
N_UNITS = FULL_PER_CORE * 4 + 1  # 25 work units per core
N_AG = FULL_PER_CORE + 1  # weight slots per core (6 full + 1 split)
KC = H // 128  # 4 contraction chunks
MC = H // 128  # 4 output-hidden chunks

LAST_RUN_INFO = {}

_CACHE = {}


def _unit_tables():
    """Per-core unit -> (agent, b0) and weight-slot tables."""
    per_core = []
    for c in range(N_CORES):
        full = list(range(c * FULL_PER_CORE, (c + 1) * FULL_PER_CORE))
        split_agent = 48 + (c // 4)
        quarter = c % 4
        units = [(a, j * BT) for a in full for j in range(4)]
        units.append((split_agent, quarter * BT))
        agents = full + [split_agent]
        per_core.append((units, agents))
    return per_core


def _build_nc():
    import concourse.bacc as bacc
    import concourse.mybir as mybir
    import concourse.tile as tile

    F32 = mybir.dt.float32
    F32R = mybir.dt.float32r
    Relu = mybir.ActivationFunctionType.Relu

    nc = bacc.Bacc(None)
    xt_d = nc.declare_dram_parameter("xt", [N_UNITS, 128, KC * BT], F32R, isOutput=False)
    w1_d = nc.declare_dram_parameter("w1", [N_AG, 128, KC * MC * 128], F32R, isOutput=False)
    w2_d = nc.declare_dram_parameter("w2", [N_AG, 128, KC * NACT], F32R, isOutput=False)
    bs_d = nc.declare_dram_parameter("bs", [N_AG, 128, 5], F32, isOutput=False)
    out_d = nc.declare_dram_parameter("out", [N_UNITS, NACT, BT], F32, isOutput=True)

    with tile.TileContext(nc) as tc:
        with (
            tc.tile_pool(name="xtp", bufs=8) as xtp,
            tc.tile_pool(name="w1p", bufs=N_AG) as w1p,
            tc.tile_pool(name="w2p", bufs=N_AG) as w2p,
            tc.tile_pool(name="bsp", bufs=N_AG) as bsp,
            tc.tile_pool(name="htp", bufs=10) as htp,
            tc.tile_pool(name="obp", bufs=4) as obp,
            tc.tile_pool(name="ps1p", bufs=6, space="PSUM") as ps1p,
            tc.tile_pool(name="ps2p", bufs=2, space="PSUM") as ps2p,
        ):
            pending = None  # (hts, w2_t, bs_t, unit) awaiting matmul2

            def flush(pending):
                hts, w2t, bst, u = pending
                ps2 = ps2p.tile([NACT, BT], F32, tag="ps2", name=f"ps2_{u}")
                for k in range(KC):
                    nc.tensor.matmul(
                        ps2,
                        lhsT=w2t[:, k * NACT : (k + 1) * NACT],
                        rhs=hts[k],
                        start=(k == 0),
                        stop=(k == KC - 1),
                    )
                ob = obp.tile([NACT, BT], F32, tag="ob", name=f"ob_{u}")
                nc.vector.tensor_scalar_add(out=ob, in0=ps2, scalar1=bst[:NACT, 4:5])
                nc.gpsimd.dma_start(out=out_d[u], in_=ob)

            # Preload all agents' weights up-front: keeps the Sync DMA FIFO
            # free of slot-release waits (it carries only the xt stream), so
            # agent switches never stall the PE on a demand weight fetch.
            w1_ts, w2_ts, bs_ts = [], [], []
            for ai in range(N_AG):
                # Agent 0's w1 rides the fast Sync queue ahead of the xt
                # stream; everything else trickles in via SWDGE (gpsimd) so
                # it never head-of-line blocks the xt loads.
                w1_t = w1p.tile([128, KC * MC * 128], F32R, tag="w1", name=f"w1_{ai}")
                (nc.sync if ai == 0 else nc.gpsimd).dma_start(out=w1_t, in_=w1_d[ai])
                w2_t = w2p.tile([128, KC * NACT], F32R, tag="w2", name=f"w2_{ai}")
                nc.gpsimd.dma_start(out=w2_t, in_=w2_d[ai])
                bs_t = bsp.tile([128, 5], F32, tag="bs", name=f"bs_{ai}")
                nc.gpsimd.dma_start(out=bs_t, in_=bs_d[ai])
                w1_ts.append(w1_t)
                w2_ts.append(w2_t)
                bs_ts.append(bs_t)

            for u in range(N_UNITS):
                ai = u // 4 if u < FULL_PER_CORE * 4 else FULL_PER_CORE
                w1_t, w2_t, bs_t = w1_ts[ai], w2_ts[ai], bs_ts[ai]

                xt_t = xtp.tile([128, KC * BT], F32R, tag="xt", name=f"xt_{u}")
                nc.sync.dma_start(out=xt_t, in_=xt_d[u])

                hts = []
                for m in range(MC):
                    ps1 = ps1p.tile([128, BT], F32, tag="ps1", name=f"ps1_{u}_{m}")
                    for k in range(KC):
                        nc.tensor.matmul(
                            ps1,
                            lhsT=w1_t[:, (k * MC + m) * 128 : (k * MC + m + 1) * 128],
                            rhs=xt_t[:, k * BT : (k + 1) * BT],
                            start=(k == 0),
                            stop=(k == KC - 1),
                        )
                    ht = htp.tile([128, BT], F32R, tag="ht", name=f"ht_{u}_{m}")
                    nc.scalar.activation(out=ht, in_=ps1, func=Relu, bias=bs_t[:, m : m + 1])
                    hts.append(ht)

                if pending is not None:
                    flush(pending)
                pending = (hts, w2_t, bs_t, u)
            flush(pending)

    nc.finalize()
    return nc


def _prep_inputs(x_in, W1, b1, W2, b2, routing):
    """Host-side: routing gather, onehot fold, per-core tiling/layout."""
    x_in = np.ascontiguousarray(x_in, dtype=np.float32)
    W1 = np.asarray(W1, dtype=np.float32)
    b1 = np.asarray(b1, dtype=np.float32)
    W2 = np.asarray(W2, dtype=np.float32)
    b2 = np.asarray(b2, dtype=np.float32)
    routing = np.asarray(routing)

    W1r = W1[routing]  # [A, H+A, H]
    W2r = W2[routing]  # [A, H, NACT]
    bias1e = b1[routing] + W1r[np.arange(A), H + np.arange(A), :]  # [A, H]
    b2e = b2[routing]  # [A, NACT]

    # Per-agent device layouts
    # w1 slot: [128, KC*MC*128]; [p, (k*MC+m)*128+c] = W1r[a, k*128+p, m*128+c]
    w1_all = (
        W1r[:, :H, :]
        .reshape(A, KC, 128, MC, 128)
        .transpose(0, 2, 1, 3, 4)
        .reshape(A, 128, KC * MC * 128)
    )
    # w2 slot: [128, KC*NACT]; [p, k*NACT+n] = W2r[a, k*128+p, n]
    w2_all = W2r.reshape(A, KC, 128, NACT).transpose(0, 2, 1, 3).reshape(A, 128, KC * NACT)
    # bias slot: [128, 5]; [:, m] = bias1e chunk m; [:NACT, 4] = b2e
    bs_all = np.zeros((A, 128, 5), dtype=np.float32)
    bs_all[:, :, :4] = bias1e.reshape(A, MC, 128).transpose(0, 2, 1)
    bs_all[:, :NACT, 4] = b2e

    per_core = _unit_tables()
    in_maps = []
    for c in range(N_CORES):
        units, agents = per_core[c]
        xt = np.empty((N_UNITS, 128, KC * BT), dtype=np.float32)
        for u, (a, b0) in enumerate(units):
            # [p, k*BT+b] = x_in[b0+b, a, k*128+p]
            xs = x_in[b0 : b0 + BT, a, :]  # [BT, H]
            xt[u] = xs.T.reshape(KC, 128, BT).transpose(1, 0, 2).reshape(128, KC * BT)
        in_maps.append(
            {
                "xt": xt,
                "w1": np.ascontiguousarray(w1_all[agents]),
                "w2": np.ascontiguousarray(w2_all[agents]),
                "bs": np.ascontiguousarray(bs_all[agents]),
            }
        )
    return in_maps, per_core


def _install_ntff_hook():
    import types

    try:
        from antenv.axon_hooks import get_axon_ntff_profile_hook  # noqa: F401

        return
    except ImportError:
        pass
    try:
        import antenv
        from trn_agent_boot.trn_boot import _ntff_profile_via_ctypes

        hook = _ntff_profile_via_ctypes("/opt/axon/libaxon_pjrt.so")
        mod = types.ModuleType("antenv.axon_hooks")
        mod.get_axon_ntff_profile_hook = lambda: hook
        mod.set_axon_ntff_profile_hook = lambda h: None
        sys.modules["antenv.axon_hooks"] = mod
        antenv.axon_hooks = mod
    except Exception:
        pass


def kernel(x_in, W1, b1, W2, b2, routing):
    from concourse.bass_utils import run_bass_kernel_spmd

    trace = bool(os.environ.get("TRN_KERNEL_TRACE"))
    if trace:
        _install_ntff_hook()

    if "nc" not in _CACHE:
        _CACHE["nc"] = _build_nc()
    nc = _CACHE["nc"]

    in_maps, per_core = _prep_inputs(x_in, W1, b1, W2, b2, routing)

    kwargs = {}
    if trace:
        kwargs = dict(trace=True, tmpdir=os.environ.get("TRN_KERNEL_TRACE_DIR") or None)
    res = run_bass_kernel_spmd(nc, in_maps, core_ids=list(range(N_CORES)), **kwargs)

    LAST_RUN_INFO.clear()
    LAST_RUN_INFO["exec_time_ns"] = res.exec_time_ns
    LAST_RUN_INFO["results"] = res

    out_full = np.empty((B, A, NACT), dtype=np.float32)
    for c in range(N_CORES):
        units, _ = per_core[c]
        oc = res.results[c]["out"]  # [N_UNITS, NACT, BT]
        for u, (a, b0) in enumerate(units):
            out_full[b0 : b0 + BT, a, :] = oc[u].T
    return out_full
